# revision 9
# baseline (speedup 1.0000x reference)
"""Trainium2 Bass kernel for nn_Coords2Stress (batched Kirchhoff matrices).

Math per sample (N=2048 atoms, n=num_atoms valid):
  c       = coords.reshape(N, 3), zeroed for padded atoms
  d2[i,j] = |ci - cj|^2, zeroed when i or j invalid
  A       = -exp(-sqrt(d2))          (padded pairs -> -1)
  K       = A with diag replaced by -rowsum(A) on valid rows, -1 on invalid

Device strategy: pure data parallel, 2 samples per core on 8 cores.
K is symmetric, so only the block-upper-triangle is computed directly;
the lower triangle is produced by PE transposes of finished tiles.

Per sample, 16 row stripes of [128, 2048]. For stripe rb:
  direct cols [rb*128, 2048):
    d2 via augmented Gram matmul on TensorE (K=8 contraction, fp32):
      L = [x, y, z, r, v, 0, 0, 0] (per-atom col, zeroed when invalid)
      R = [-2x, -2y, -2z, v, r, 0, 0, 0];  d2 = L.T @ R
    DVE relu-drain PSUM->SBUF (clamps fp32 cancellation negatives; diag
      128-block additionally multiplied by (1-I) to force exact zeros)
    ACT sqrt in-place, ACT exp(-x) in-place with accum_out row sums
    DVE negate in-place
  mirror cols [0, rb*128): PE-transpose finished 128-blocks from earlier
    stripes -> PSUM, DVE copy to stripe buffer with accum_out (row sums)
  diagonal: K[i,i] = valid_i * (sum_j exp(-d_ij)) via eye-masked predicated
    copy (invalid rows keep -1)
  one 1MB DMA out per stripe.

ACT table sets are batched: all 16 sqrts (one table load), then the exp
phase (one load) interleaved with per-stripe finalization so DMA flows.
"""
import numpy as np

import concourse.bass as bass
import concourse.tile as tile
from concourse import bacc, mybir
from concourse import bass_utils

B, N3 = 16, 6144
N = 2048
P = 128
NCORES = 8
SPC = B // NCORES          # samples per core
NRB = N // P               # row blocks per sample
FP = mybir.dt.float32
ALU = mybir.AluOpType
AF = mybir.ActivationFunctionType

_cache = {}


def _build_bass():
    nc = bacc.Bacc("TRN2", target_bir_lowering=False, debug=False,
                   enable_asserts=False, num_devices=NCORES)

    L = nc.dram_tensor("L", [SPC, 8, N], FP, kind="ExternalInput")
    R = nc.dram_tensor("R", [SPC, 8, N], FP, kind="ExternalInput")
    VM = nc.dram_tensor("VM", [P, SPC * NRB], FP, kind="ExternalInput")
    EYE = nc.dram_tensor("EYE", [P, P], FP, kind="ExternalInput")
    OMI = nc.dram_tensor("OMI", [P, P], FP, kind="ExternalInput")
    EYEI = nc.dram_tensor("EYEI", [P, P], mybir.dt.uint8, kind="ExternalInput")
    OUT = nc.dram_tensor("OUT", [SPC, N, N], FP, kind="ExternalOutput")

    with tile.TileContext(nc, trace_sim=False) as tc:
        from concourse.tile_rust import add_dep_helper
        with tc.tile_pool(name="const", bufs=1) as cpool, \
             tc.tile_pool(name="stripes", bufs=1) as spool_big, \
             tc.tile_pool(name="stripes2", bufs=2) as spool_big2, \
             tc.tile_pool(name="small", bufs=12) as spool, \
             tc.tile_pool(name="psum", bufs=3, space="PSUM") as ppool, \
             tc.tile_pool(name="tpsum", bufs=2, space="PSUM") as tpool:

            lt = cpool.tile([8, SPC * N], FP, tag="lt")
            rt = cpool.tile([8, SPC * N], FP, tag="rt")
            vmt = cpool.tile([P, SPC * NRB], FP, tag="vmt")
            eye = cpool.tile([P, P], FP, tag="eye")
            omi = cpool.tile([P, P], FP, tag="omi")
            eyei = cpool.tile([P, P], mybir.dt.uint8, tag="eyei")
            for s in range(SPC):
                nc.sync.dma_start(lt[:, s * N:(s + 1) * N], L.ap()[s])
                nc.sync.dma_start(rt[:, s * N:(s + 1) * N], R.ap()[s])
            nc.sync.dma_start(vmt[:], VM.ap())
            nc.sync.dma_start(eye[:], EYE.ap())
            nc.sync.dma_start(omi[:], OMI.ap())
            nc.sync.dma_start(eyei[:], EYEI.ap())

            prev_last_exp = None
            for s in range(SPC):
                S = {}    # stripe buffers
                for (g0, g1) in ((0, 4), (4, NRB)):
                    sqrt_insts = []
                    exp_insts = []
                    # -- phase 1: matmuls + relu drains + group sqrts (one table set)
                    for rb in range(g0, g1):
                        d0, d1 = rb * P, (rb + 1) * P
                        u = (spool_big2 if rb < 4 else spool_big).tile(
                            [P, N], FP, tag=f"st{rb}")
                        for h0 in (0, 1024):
                            h1 = h0 + 1024
                            if h1 <= d0:
                                continue        # half entirely left of direct region
                            c_lo = max(d0, h0)
                            pt = ppool.tile([P, 1024], FP, tag="pt")
                            for bk in range(c_lo // 512, h1 // 512):
                                c0 = max(c_lo, bk * 512)
                                c1 = (bk + 1) * 512
                                nc.tensor.matmul(
                                    pt[:, c0 - h0:c1 - h0],
                                    lt[:, s * N + d0: s * N + d1],
                                    rt[:, s * N + c0: s * N + c1],
                                    start=True, stop=True)
                            if h0 <= d0 < h1:
                                # diag block: relu then zero diagonal via (1-I)
                                nc.vector.scalar_tensor_tensor(
                                    u[:, d0:d1], pt[:, d0 - h0:d1 - h0], 0.0,
                                    omi[:], ALU.max, ALU.mult)
                                if d1 < h1:
                                    nc.vector.tensor_scalar(
                                        u[:, d1:h1], pt[:, d1 - h0:1024], 0.0,
                                        None, ALU.max)
                            else:
                                nc.vector.tensor_scalar(
                                    u[:, h0:h1], pt[:, 0:1024], 0.0, None,
                                    ALU.max)
                        si = nc.scalar.activation(u[:, d0:N], u[:, d0:N], AF.Sqrt)
                        sqrt_insts.append(si)
                        S[rb] = u
                    # -- phase 2: per stripe: exp, negate, mirrors, diag, DMA
                    for rb in range(g0, g1):
                        d0, d1 = rb * P, (rb + 1) * P
                        u = S[rb]
                        a = spool.tile([P, 1], FP, tag="acc")
                        ei = nc.scalar.activation(u[:, d0:N], u[:, d0:N], AF.Exp,
                                                  scale=-1.0, accum_out=a[:])
                        exp_insts.append(ei)
                        nc.gpsimd.tensor_scalar(u[:, d0:N], u[:, d0:N], -1.0,
                                                None, ALU.mult)
                        # mirrors: transpose finished blocks (cb, rb) cb<rb
                        macc = []
                        for c0 in range(0, d0, 512):
                            wc = min(512, d0 - c0)
                            tp = tpool.tile([P, 512], FP, tag="tp")
                            for bi in range(wc // P):
                                cb = c0 // P + bi
                                nc.tensor.transpose(
                                    tp[:, bi * P:(bi + 1) * P],
                                    S[cb][:, d0:d1], eye[:])
                            m = spool.tile([P, 1], FP, tag="macc")
                            nc.vector.tensor_scalar(
                                u[:, c0:c0 + wc], tp[:, 0:wc], 0.0, 0.0,
                                ALU.add, ALU.add, accum_out=m[:])
                            macc.append(m)
                        # dv = valid * (acc_direct - sum(mirror accums));
                        # mirror accums hold sums of negated values
                        t = a
                        for m in macc:
                            t2 = spool.tile([P, 1], FP, tag="tsub")
                            nc.vector.tensor_tensor(t2[:], t[:], m[:],
                                                    op=ALU.subtract)
                            t = t2
                        dv = spool.tile([P, 1], FP, tag="dv")
                        nc.vector.tensor_tensor(
                            dv[:], t[:], vmt[:, s * NRB + rb: s * NRB + rb + 1],
                            op=ALU.mult)
                        # diag currently -1 exactly; add eye*(dv+valid):
                        # valid rows: -1 + dv + 1 = dv; invalid: unchanged -1
                        dvp = spool.tile([P, 1], FP, tag="dvp")
                        nc.vector.tensor_tensor(
                            dvp[:], dv[:], vmt[:, s * NRB + rb: s * NRB + rb + 1],
                            op=ALU.add)
                        nc.vector.scalar_tensor_tensor(
                            u[:, d0:d1], eye[:], dvp[:], u[:, d0:d1],
                            ALU.mult, ALU.add)
                        nc.sync.dma_start(OUT.ap()[s, d0:d1, :], u[:])
                    # -- ACT table-set phase ordering (no-sync scheduler edges)
                    add_dep_helper(exp_insts[0].ins, sqrt_insts[-1].ins, False,
                                   "act table batching: exp after group sqrts")
                    if prev_last_exp is not None:
                        add_dep_helper(sqrt_insts[0].ins, prev_last_exp.ins, False,
                                       "act table batching: sqrt after prev exps")
                    prev_last_exp = exp_insts[-1]
    nc.compile()
    return nc


def _prep_inputs(coords: np.ndarray, num_atoms: np.ndarray):
    """Host-side layout prep: build augmented Gram operands per sample."""
    c = coords.reshape(B, N, 3).astype(np.float32)
    ar = np.arange(N)
    valid = (ar[None, :] < num_atoms[:, None])          # [B, N] bool
    cm = np.where(valid[..., None], c, 0.0).astype(np.float32)
    r = (cm * cm).sum(-1).astype(np.float32)             # [B, N]
    vf = valid.astype(np.float32)
    Lm = np.zeros((B, 8, N), np.float32)
    Rm = np.zeros((B, 8, N), np.float32)
    xT = np.transpose(cm, (0, 2, 1))                     # [B, 3, N]
    Lm[:, 0:3] = xT
    Lm[:, 3] = r * vf
    Lm[:, 4] = vf
    Rm[:, 0:3] = -2.0 * xT
    Rm[:, 3] = vf
    Rm[:, 4] = r * vf
    return Lm, Rm, vf


def kernel(coords: np.ndarray, num_atoms: np.ndarray) -> np.ndarray:
    if "nc" not in _cache:
        _cache["nc"] = _build_bass()
    nc = _cache["nc"]

    Lm, Rm, vm = _prep_inputs(coords, num_atoms)
    eye = np.eye(P, dtype=np.float32)
    omi = (1.0 - eye).astype(np.float32)

    in_maps = []
    for core in range(NCORES):
        sl = slice(core * SPC, (core + 1) * SPC)
        vmc = np.zeros((P, SPC * NRB), np.float32)
        for s in range(SPC):
            for rb in range(NRB):
                vmc[:, s * NRB + rb] = vm[core * SPC + s, rb * P:(rb + 1) * P]
        in_maps.append({
            "L": np.ascontiguousarray(Lm[sl]),
            "R": np.ascontiguousarray(Rm[sl]),
            "VM": vmc,
            "EYE": eye,
            "OMI": omi,
            "EYEI": eye.astype(np.uint8),
        })

    res = bass_utils.run_bass_kernel_spmd(nc, in_maps, core_ids=list(range(NCORES)))
    out = np.concatenate([res.results[c]["OUT"] for c in range(NCORES)], axis=0)
    return out.astype(np.float32)
